# revision 1
# baseline (speedup 1.0000x reference)
"""Trainium2 Bass kernel for EventDiffusion GNN (GCNConv + GATConv, 2 layers).

Sharding: nodes partitioned into 8 contiguous ranges (one per NeuronCore).
Each core aggregates messages for its destination-node range; layer-1 hidden
states are exchanged with an AllGather so every core can gather arbitrary
source rows for layer 2.

Device dataflow per layer:
  - feature table (X@W) computed on every core (replicated matmul, fp32r)
    and written to a per-core DRAM table
  - per-edge rows gathered from the table with HW dma_gather (int16 indices)
  - segment-sum by destination done as one-hot matmuls accumulating in PSUM:
    for each tile of 128 edges, M[e, j] = coeff_e * (dstslot_e == j) is built
    with one tensor_scalar (iota==dslot)*coeff op, then PSUM += M^T @ G
  - GAT softmax: alpha_e = exp(e_e - eself[dst_e]) unnormalized, denominator
    accumulated via an all-ones table column; per-dst shift by the self-loop
    logit is mathematically exact and numerically safe (clamped at +80)
"""

import numpy as np

import concourse.bass as bass
import concourse.bacc as bacc
import concourse.mybir as mybir
import concourse.tile as tile
from concourse.bass_utils import run_bass_kernel_spmd

FP32 = mybir.dt.float32
FP32R = mybir.dt.float32r
BF16 = mybir.dt.bfloat16
I16 = mybir.dt.int16

N_CORES = 8
D = 256
W2COLS = 320  # 256 feats | 256:v1(asrc) | 257:ones | 258:v2(adst) | pad
ACOLS = 64    # by-dst gather width (table2 cols 256:320)

# table dtype: float32r = fp32 with 11-bit mantissa (TF32-like), full-rate PE
TDT = FP32R


def _round_f32r(a):
    """Round-to-nearest-even fp32 -> fp32r (low 12 mantissa bits zeroed)."""
    u = np.ascontiguousarray(a, np.float32).view(np.uint32)
    lsb = (u >> np.uint32(12)) & np.uint32(1)
    r = (u + np.uint32(0x7FF) + lsb) & np.uint32(0xFFFFF000)
    return r.view(np.float32)


def _pad_nodes(n):
    # NPAD must be a multiple of 128*N_CORES so each core owns NPAD/8 = 128*k
    return -(-n // (128 * N_CORES)) * (128 * N_CORES)


# ----------------------------------------------------------------------------
# host-side preprocessing (sharding + index/metadata construction)
# ----------------------------------------------------------------------------

def _prep(event_emb, edge_index, W1, b1, W2, att_src, att_dst, b2):
    X = np.ascontiguousarray(np.asarray(event_emb, np.float32))
    n = X.shape[0]
    npad = _pad_nodes(n)
    per = npad // N_CORES
    nblk = per // 128

    ei = np.asarray(edge_index, np.int64)
    src = np.concatenate([ei[0], np.arange(n, dtype=np.int64)])
    dst = np.concatenate([ei[1], np.arange(n, dtype=np.int64)])
    deg = np.bincount(dst, minlength=n).astype(np.float32)
    dinv = np.where(deg > 0, 1.0 / np.sqrt(deg), 0.0).astype(np.float32)
    coeff = (dinv[src] * dinv[dst]).astype(np.float32)

    order = np.argsort(dst, kind="stable")
    src, dst, coeff = src[order], dst[order], coeff[order]

    core_of = dst // per
    loc_blk = (dst % per) // 128

    counts = np.zeros((N_CORES, nblk), np.int64)
    np.add.at(counts, (core_of, loc_blk), 1)
    T = [max(1, int(-(-counts[:, b].max() // 128))) for b in range(nblk)]

    # split edge arrays per (core, block)
    key = core_of * nblk + loc_blk
    korder = np.argsort(key, kind="stable")
    src, dst, coeff = src[korder], dst[korder], coeff[korder]
    bounds = np.searchsorted(key[korder], np.arange(N_CORES * nblk + 1))

    def wrap16(idx):
        s = idx.astype(np.int16).reshape(-1, 16).T  # [16, S]
        return np.tile(s, (8, 1))  # [128, S]

    ngb = npad // 128  # number of src blocks
    per_core = []
    for c in range(N_CORES):
        idxs_l, idxd_l, dsl_l = [], [], []
        # layer-1 push matrices: m1[b, g, srcslot, dstslot] = sum of coeffs
        m1 = np.zeros((nblk, ngb, 128, 128), np.float32)
        for b in range(nblk):
            lo, hi = bounds[c * nblk + b], bounds[c * nblk + b + 1]
            s, d_, co = src[lo:hi], dst[lo:hi], coeff[lo:hi]
            np.add.at(m1[b], (s // 128, s % 128, d_ % 128), co)
            L = T[b] * 128
            pad = L - len(s)
            s = np.concatenate([s, np.zeros(pad, np.int64)])
            dglob = np.concatenate([d_, np.zeros(pad, np.int64)])
            dl = np.concatenate(
                [d_ - c * per - b * 128, np.full(pad, -1, np.int64)]
            ).astype(np.float32)
            idxs_l.append(wrap16(s))
            idxd_l.append(wrap16(dglob))
            dsl_l.append(dl.reshape(T[b], 128).T)  # [128, T[b]]
        per_core.append(
            dict(
                idxs=np.ascontiguousarray(np.concatenate(idxs_l, axis=1)),
                idxd=np.ascontiguousarray(np.concatenate(idxd_l, axis=1)),
                dslot=np.ascontiguousarray(np.concatenate(dsl_l, axis=1)),
                m1=_round_f32r(m1),
            )
        )

    # shared (replicated) arrays
    W1 = np.asarray(W1, np.float32)
    W2 = np.asarray(W2, np.float32)
    b1 = np.asarray(b1, np.float32)
    b2 = np.asarray(b2, np.float32)
    v1 = (W2 @ np.asarray(att_src, np.float32)).astype(np.float32)
    v2 = (W2 @ np.asarray(att_dst, np.float32)).astype(np.float32)

    Xp = np.zeros((npad, D), np.float32)
    Xp[:n] = X
    xt = _round_f32r(Xp.T.reshape(2, 128, npad))

    w1k = _round_f32r(W1.reshape(2, 128, D))
    W2p = np.zeros((D, W2COLS), np.float32)
    W2p[:, :D] = W2
    W2p[:, 256] = v1
    W2p[:, 258] = v2
    w2k = _round_f32r(W2p.reshape(2, 128, W2COLS))

    shared = dict(
        xt=xt,
        w1=w1k,
        w2p=w2k,
        b1b=np.ascontiguousarray(np.tile(b1[None, :], (128, 1))),
        b2b=np.ascontiguousarray(np.tile(b2[None, :], (128, 1))),
        ones320=np.ascontiguousarray(
            np.tile(
                np.eye(1, W2COLS, 257, dtype=np.float32), (128, 1)
            )
        ),
        iota=np.ascontiguousarray(
            np.tile(np.arange(128, dtype=np.float32)[None, :], (128, 1))
        ),
        ident=np.eye(128, dtype=np.float32),
    )
    return shared, per_core, T, n, npad, per, nblk


# ----------------------------------------------------------------------------
# device program
# ----------------------------------------------------------------------------

def _mm_dt(ap):
    """matmul operand dtype: full-rate fp32 via float32r bitcast."""
    if ap.dtype == FP32:
        return ap.bitcast(FP32R)
    return ap


def _build_nc(T, npad, per, nblk, use_collective=True):
    st = sum(T)
    si = 8 * st
    nc = bacc.Bacc(
        "TRN2", target_bir_lowering=False, debug=False, num_devices=N_CORES
    )

    # I/O
    xt_d = nc.dram_tensor("xt", [2, 128, npad], TDT, kind="ExternalInput")
    w1_d = nc.dram_tensor("w1", [2, 128, D], TDT, kind="ExternalInput")
    w2_d = nc.dram_tensor("w2p", [2, 128, W2COLS], TDT, kind="ExternalInput")
    b1_d = nc.dram_tensor("b1b", [128, D], FP32, kind="ExternalInput")
    b2_d = nc.dram_tensor("b2b", [128, D], FP32, kind="ExternalInput")
    ones_d = nc.dram_tensor("ones320", [128, W2COLS], FP32, kind="ExternalInput")
    iota_d = nc.dram_tensor("iota", [128, 128], FP32, kind="ExternalInput")
    ident_d = nc.dram_tensor("ident", [128, 128], FP32, kind="ExternalInput")
    idxs_d = nc.dram_tensor("idxs", [128, si], I16, kind="ExternalInput")
    idxd_d = nc.dram_tensor("idxd", [128, si], I16, kind="ExternalInput")
    dsl_d = nc.dram_tensor("dslot", [128, st], FP32, kind="ExternalInput")
    m1_d = nc.dram_tensor(
        "m1", [nblk, npad // 128, 128, 128], TDT, kind="ExternalInput"
    )
    out_d = nc.dram_tensor("out_slice", [per, D], FP32, kind="ExternalOutput")

    # internal DRAM
    table2 = nc.dram_tensor("table2", [npad, W2COLS], TDT)
    ht_slice = nc.dram_tensor("ht_slice", [2, 128, per], TDT)
    ht_full = nc.dram_tensor(
        "ht_full", [N_CORES, 2, 128, per], TDT, addr_space="Shared"
    )

    eq, mu, ad = (
        mybir.AluOpType.is_equal,
        mybir.AluOpType.mult,
        mybir.AluOpType.max,
    )

    with tile.TileContext(nc) as tc:
        with tc.tile_pool(name="const", bufs=1) as cp:
            iota_sb = cp.tile([128, 128], FP32)
            nc.sync.dma_start(iota_sb[:], iota_d[:, :])
            ident_sb = cp.tile([128, 128], FP32)
            nc.sync.dma_start(ident_sb[:], ident_d[:, :])
            b1_sb = cp.tile([128, D], FP32)
            nc.sync.dma_start(b1_sb[:], b1_d[:, :])
            b2_sb = cp.tile([128, D], FP32)
            nc.sync.dma_start(b2_sb[:], b2_d[:, :])
            ones_sb = cp.tile([128, W2COLS], FP32)
            nc.sync.dma_start(ones_sb[:], ones_d[:, :])
            idxs_sb = cp.tile([128, si], I16)
            nc.sync.dma_start(idxs_sb[:], idxs_d[:, :])
            idxd_sb = cp.tile([128, si], I16)
            nc.sync.dma_start(idxd_sb[:], idxd_d[:, :])
            dsl_sb = cp.tile([128, st], FP32)
            nc.sync.dma_start(dsl_sb[:], dsl_d[:, :])
            w1_sb = cp.tile([128, 2, D], TDT)
            w2_sb = cp.tile([128, 2, W2COLS], TDT)
            for k in range(2):
                nc.sync.dma_start(w1_sb[:, k, :], w1_d[k])
                nc.sync.dma_start(w2_sb[:, k, :], w2_d[k])

            # -------- phase 1A: XW1 = X @ W1, kept resident in SBUF --------
            ngb = npad // 128
            half = npad // 2
            with (
                tc.tile_pool(name="xw1_p", bufs=1) as xwp,
                tc.tile_pool(name="xt_p", bufs=1) as xp,
                tc.tile_pool(name="m1s_p", bufs=3) as mp,
                tc.tile_pool(name="h1_p", bufs=3) as hp,
                tc.tile_pool(name="ht_p", bufs=1) as htp,
                tc.psum_pool(name="ps1_p", bufs=2) as pp,
                tc.psum_pool(name="ps2_p", bufs=2) as pa,
                tc.psum_pool(name="pt_p", bufs=2) as pt,
            ):
                xw1_sb = xwp.tile([128, ngb, D], TDT)
                for hh in range(2):
                    xt_sb = xp.tile([128, 2, half], TDT, tag="xt")
                    for k in range(2):
                        nc.sync.dma_start(
                            xt_sb[:, k, :], xt_d[k, :, hh * half:(hh + 1) * half]
                        )
                    for j in range(half // 128):
                        g = hh * (half // 128) + j
                        ps = pp.tile([128, D], FP32, tag="ps1")
                        for k in range(2):
                            nc.tensor.matmul(
                                ps[:],
                                lhsT=xt_sb[:, k, j * 128:(j + 1) * 128],
                                rhs=w1_sb[:, k, :],
                                start=(k == 0),
                                stop=(k == 1),
                            )
                        nc.vector.tensor_copy(xw1_sb[:, g, :], ps[:])

                # -------- phase 1B: GCN aggregate (push mode) + H^T --------
                GC = 4  # src blocks per m1 stream tile
                ht_st = htp.tile([128, 2, per], TDT)
                for b in range(nblk):
                    psa = pa.tile([128, D], FP32, tag="agg1")
                    for gg in range(0, ngb, GC):
                        mt = mp.tile([128, GC, 128], TDT, tag="m1s")
                        nc.sync.dma_start(
                            mt[:],
                            m1_d[b, gg:gg + GC].rearrange("g s d -> s g d"),
                        )
                        for j in range(GC):
                            g = gg + j
                            nc.tensor.matmul(
                                psa[:],
                                lhsT=mt[:, j, :],
                                rhs=xw1_sb[:, g, :],
                                start=(g == 0),
                                stop=(g == ngb - 1),
                            )
                    hs = hp.tile([128, D], FP32, tag="h1")
                    nc.vector.tensor_tensor(
                        hs[:], psa[:], b1_sb[:], op=mybir.AluOpType.add
                    )
                    nc.vector.tensor_scalar_max(hs[:], hs[:], 0.0)
                    for k in range(2):
                        ptt = pt.tile([128, 128], FP32, tag="pt")
                        nc.tensor.transpose(
                            ptt[:], hs[:, k * 128:(k + 1) * 128], ident_sb[:]
                        )
                        nc.vector.tensor_copy(
                            ht_st[:, k, b * 128:(b + 1) * 128], ptt[:]
                        )
                for k in range(2):
                    nc.sync.dma_start(ht_slice[k], ht_st[:, k, :])

            if use_collective:
                nc.gpsimd.collective_compute(
                    "AllGather",
                    mybir.AluOpType.bypass,
                    replica_groups=[list(range(N_CORES))],
                    ins=[ht_slice[:, :, :]],
                    outs=[ht_full[:, :, :, :]],
                )
            else:
                # debug fallback: every rank slot gets the local slice
                for r in range(N_CORES):
                    nc.sync.dma_start(ht_full[r], ht_slice[:, :, :])

            # ---------------- phase 2A: table2 = H @ [W2|v1|1|v2] ----------
            with (
                tc.tile_pool(name="ht2_p", bufs=1) as hp2,
                tc.tile_pool(name="st2_p", bufs=3) as sp2,
                tc.psum_pool(name="ps3_p", bufs=2) as pp,
            ):
                ht_sb = hp2.tile([128, 2 * N_CORES, per], TDT)
                for r in range(N_CORES):
                    for k in range(2):
                        nc.sync.dma_start(ht_sb[:, 2 * r + k, :], ht_full[r, k])
                for g in range(npad // 128):
                    r, j = divmod(g, nblk)
                    ps = pp.tile([128, W2COLS], FP32, tag="ps3")
                    for k in range(2):
                        nc.tensor.matmul(
                            ps[:],
                            lhsT=_mm_dt(
                                ht_sb[:, 2 * r + k, j * 128:(j + 1) * 128]
                            ),
                            rhs=_mm_dt(w2_sb[:, k, :]),
                            start=(k == 0),
                            stop=(k == 1),
                        )
                    st2 = sp2.tile([128, W2COLS], TDT, tag="st2")
                    nc.vector.tensor_tensor(
                        st2[:], ps[:], ones_sb[:], op=mybir.AluOpType.add
                    )
                    nc.sync.dma_start(table2[g * 128:(g + 1) * 128, :], st2[:])

            # ---------------- phase 2B: GAT aggregate ----------------------
            with (
                tc.tile_pool(name="g2_p", bufs=2) as gp2,
                tc.tile_pool(name="a2_p", bufs=2) as ap2,
                tc.tile_pool(name="sc_p", bufs=2) as scp,
                tc.tile_pool(name="m2_p", bufs=4) as mp2,
                tc.tile_pool(name="o_p", bufs=3) as op_,
                tc.psum_pool(name="ps4_p", bufs=2) as pp,
            ):
                off = 0
                for b in range(nblk):
                    tb = T[b]
                    g2 = gp2.tile([128, tb, W2COLS], TDT, tag="g2")
                    nc.gpsimd.dma_gather(
                        g2[:],
                        table2[:, :],
                        idxs_sb[:, 8 * off: 8 * (off + tb)],
                        num_idxs=tb * 128,
                        num_idxs_reg=tb * 128,
                        elem_size=W2COLS,
                        single_packet=False,
                    )
                    a2 = ap2.tile([128, tb, ACOLS], TDT, tag="a2")
                    nc.gpsimd.dma_gather(
                        a2[:],
                        table2[:, 256:320],
                        idxd_sb[:, 8 * off: 8 * (off + tb)],
                        num_idxs=tb * 128,
                        num_idxs_reg=tb * 128,
                        elem_size=ACOLS,
                        elem_step=W2COLS,
                        single_packet=False,
                    )
                    # alpha chain on [128, tb]
                    t0 = scp.tile([128, tb], FP32, tag="t0")
                    nc.vector.tensor_tensor(
                        t0[:], g2[:, :, 256].bitcast(FP32), a2[:, :, 2].bitcast(FP32), op=mybir.AluOpType.add
                    )
                    e = scp.tile([128, tb], FP32, tag="e")
                    nc.vector.scalar_tensor_tensor(
                        e[:], t0[:], 0.2, t0[:], op0=mu, op1=ad
                    )
                    t1 = scp.tile([128, tb], FP32, tag="t1")
                    nc.vector.tensor_tensor(
                        t1[:], a2[:, :, 0].bitcast(FP32), a2[:, :, 2].bitcast(FP32), op=mybir.AluOpType.add
                    )
                    es = scp.tile([128, tb], FP32, tag="es")
                    nc.vector.scalar_tensor_tensor(
                        es[:], t1[:], 0.2, t1[:], op0=mu, op1=ad
                    )
                    esh = scp.tile([128, tb], FP32, tag="esh")
                    nc.vector.tensor_sub(esh[:], e[:], es[:])
                    nc.vector.tensor_scalar_min(esh[:], esh[:], 80.0)
                    al = scp.tile([128, tb], FP32, tag="al")
                    nc.scalar.activation(
                        al[:], esh[:], mybir.ActivationFunctionType.Exp
                    )
                    ps = pp.tile([128, W2COLS], FP32, tag="agg2")
                    for t in range(tb):
                        m2 = mp2.tile([128, 128], TDT, tag="m2")
                        nc.vector.tensor_scalar(
                            m2[:],
                            iota_sb[:],
                            dsl_sb[:, off + t: off + t + 1],
                            al[:, t: t + 1],
                            op0=eq,
                            op1=mu,
                        )
                        nc.tensor.matmul(
                            ps[:],
                            lhsT=_mm_dt(m2[:]),
                            rhs=_mm_dt(g2[:, t, :]),
                            start=(t == 0),
                            stop=(t == tb - 1),
                        )
                    sden = scp.tile([128, 1], FP32, tag="sden")
                    nc.vector.tensor_scalar_add(sden[:], ps[:, 257:258], 1e-16)
                    rc = scp.tile([128, 1], FP32, tag="rc")
                    nc.vector.reciprocal(rc[:], sden[:])
                    ob = op_.tile([128, D], FP32, tag="ob")
                    nc.vector.scalar_tensor_tensor(
                        ob[:], ps[:, 0:D], rc[:], b2_sb[:], op0=mu,
                        op1=mybir.AluOpType.add,
                    )
                    nc.vector.tensor_scalar_max(ob[:], ob[:], 0.0)
                    nc.sync.dma_start(out_d[b * 128:(b + 1) * 128, :], ob[:])
                    off += tb
    nc.finalize()
    return nc


# ----------------------------------------------------------------------------
# entry point
# ----------------------------------------------------------------------------

_CACHE = {}


def _get_nc(T, npad, per, nblk):
    key = (tuple(T), npad, per, nblk, TDT)
    if key not in _CACHE:
        _CACHE[key] = _build_nc(T, npad, per, nblk)
    return _CACHE[key]


def kernel(event_emb, edge_index, W1, b1, W2, att_src, att_dst, b2,
           _want_results=False, _trace=False):
    shared, per_core, T, n, npad, per, nblk = _prep(
        event_emb, edge_index, W1, b1, W2, att_src, att_dst, b2
    )
    nc = _get_nc(T, npad, per, nblk)
    in_maps = [{**shared, **per_core[c]} for c in range(N_CORES)]
    res = run_bass_kernel_spmd(
        nc, in_maps, core_ids=list(range(N_CORES)), trace=_trace
    )
    out = np.concatenate(
        [res.results[c]["out_slice"] for c in range(N_CORES)], axis=0
    )[:n]
    if _want_results:
        return out, res
    return out



# revision 9
# speedup vs baseline: 2.1821x; 2.1821x over previous
"""Trainium2 Bass kernel for EventDiffusion GNN (GCNConv + GATConv, 2 layers).

Sharding: nodes partitioned into 8 contiguous ranges (one per NeuronCore).
Each core aggregates messages for its destination-node range from replicated
tables (graph/data parallel per the sharding hint); the layer-1 output table
is exchanged with an AllGather so every core can gather arbitrary source rows
for layer 2.

Dataflow per core (all feature data bf16, PSUM accumulation fp32):
  L1 (GCN):  gather X[src] rows per edge (HW dma_gather, 4 SWDGE queues),
             scatter-sum by dst-slot via host-precomputed one-hot matrices
             P1[e, j] = coeff_e * (dslot_e == j) streamed bf16:
             psum += P1^T @ gathered.  Then h = relu(AX @ W1 + b1) via PE
             transposes + matmuls, and the layer-2 row table
             t2[d, :] = [h@W2 | h.v1 | 1 | h.v2] is built locally per block.
  comm:      AllGather of the local t2 slice (2 chunks, overlapped with L1).
  L2 (GAT):  gather t2[src] rows per edge; per-edge dst values via
             psum_dv = PT^T @ adst (PT = transposed one-hot); alpha = exp of
             unshifted logits (logits are O(1), no overflow risk; the softmax
             shift cancels exactly in numerator/denominator); one-hot scaled
             by alpha on the Scalar engine; psum += (P2*alpha)^T @ gathered.
             Denominator rides along as the all-ones table column.
"""

import numpy as np
import ml_dtypes

import concourse.bass as bass
import concourse.bacc as bacc
import concourse.mybir as mybir
import concourse.tile as tile
from concourse.bass_utils import run_bass_kernel_spmd

FP32 = mybir.dt.float32
BF16 = mybir.dt.bfloat16
I16 = mybir.dt.int16
NPBF16 = ml_dtypes.bfloat16

N_CORES = 8
D = 256
TCOLS = 384  # t2 row: [xw2(0:256) | asrc(256) | one(257) | adst(258) | pad)
NQ = 4       # SWDGE queues
GCH = 4      # gather chunks per block (round-robin over queues)


def _pad_nodes(n):
    return -(-n // (128 * N_CORES)) * (128 * N_CORES)


def _wrap16(idx):
    s = idx.astype(np.int16).reshape(-1, 16).T  # [16, L/16]
    return np.tile(s, (8, 1))  # [128, L/16]


# ----------------------------------------------------------------------------
# host-side preprocessing (graph structure only: indices + one-hot scatters)
# ----------------------------------------------------------------------------

def _prep(event_emb, edge_index, W1, b1, W2, att_src, att_dst, b2):
    X = np.ascontiguousarray(np.asarray(event_emb, np.float32))
    n = X.shape[0]
    npad = _pad_nodes(n)
    per = npad // N_CORES
    nblk = per // 128

    ei = np.asarray(edge_index, np.int64)
    src = np.concatenate([ei[0], np.arange(n, dtype=np.int64)])
    dst = np.concatenate([ei[1], np.arange(n, dtype=np.int64)])
    deg = np.bincount(dst, minlength=n).astype(np.float32)
    dinv = np.where(deg > 0, 1.0 / np.sqrt(deg), 0.0).astype(np.float32)
    coeff = (dinv[src] * dinv[dst]).astype(np.float32)

    # chunk-major row permutation matching the on-device table2 layout
    # ([chunk, rank, row]): global row g -> h*(8*half) + c*half + rr
    half = per // 2

    def _perm(g):
        c, r = g // per, g % per
        return (r // half) * (N_CORES * half) + c * half + (r % half)

    key = (dst // per) * nblk + (dst % per) // 128
    order = np.argsort(key, kind="stable")
    src, dst, coeff, key = src[order], dst[order], coeff[order], key[order]
    srcp = _perm(src)
    bounds = np.searchsorted(key, np.arange(N_CORES * nblk + 1))
    cnt = (bounds[1:] - bounds[:-1]).reshape(N_CORES, nblk)
    T = [max(1, int(-(-cnt[:, b].max() // 128))) for b in range(nblk)]
    offs = np.concatenate([[0], np.cumsum(T)]).astype(np.int64)
    st = int(offs[-1])

    per_core = []
    for c in range(N_CORES):
        idxs = np.zeros((128, 8 * st), np.int16)
        p1 = np.zeros((128, st, 128), NPBF16)
        p2 = np.zeros((128, st, 128), NPBF16)
        pt = np.zeros((128, st, 128), NPBF16)
        for b in range(nblk):
            lo, hi = bounds[c * nblk + b], bounds[c * nblk + b + 1]
            s, d_, co = srcp[lo:hi], dst[lo:hi], coeff[lo:hi]
            m = hi - lo
            L = T[b] * 128
            e = np.arange(m)
            t = offs[b] + e // 128
            p = e % 128
            j = (d_ - (c * per + b * 128)).astype(np.int64)
            p1[p, t, j] = co.astype(NPBF16)
            p2[p, t, j] = NPBF16(1.0)
            pt[j, t, p] = NPBF16(1.0)
            sfull = np.zeros(L, np.int64)
            sfull[:m] = s
            idxs[:, 8 * offs[b]: 8 * (offs[b] + T[b])] = _wrap16(sfull)
        per_core.append(dict(idxs=idxs, p1=p1, p2=p2, pt=pt))

    W1 = np.asarray(W1, np.float32)
    W2 = np.asarray(W2, np.float32)
    v1 = W2 @ np.asarray(att_src, np.float32)
    v2 = W2 @ np.asarray(att_dst, np.float32)

    Xp = np.zeros((npad, D), NPBF16)
    Xp[_perm(np.arange(n))] = X.astype(NPBF16)

    W2p = np.zeros((D, TCOLS), np.float32)
    W2p[:, :D] = W2
    W2p[:, 256] = v1
    W2p[:, 258] = v2

    ones384 = np.zeros((128, TCOLS), np.float32)
    ones384[:, 257] = 1.0

    shared = dict(
        xtab=Xp,
        w1=np.ascontiguousarray(W1.reshape(2, 128, D).astype(NPBF16)),
        w2p=np.ascontiguousarray(W2p.reshape(2, 128, TCOLS).astype(NPBF16)),
        b1r=np.ascontiguousarray(
            np.asarray(b1, np.float32).reshape(2, 128, 1)
        ),
        b2b=np.ascontiguousarray(
            np.tile(np.asarray(b2, np.float32)[None, :], (128, 1))
        ),
        ones384=ones384,
        ident=np.eye(128, dtype=NPBF16),
    )
    return shared, per_core, T, n, npad, per, nblk


# ----------------------------------------------------------------------------
# device program
# ----------------------------------------------------------------------------

def _build_nc(T, npad, per, nblk):
    st = sum(T)
    offs = np.concatenate([[0], np.cumsum(T)]).astype(np.int64)
    half = per // 2  # rows per collective chunk
    hblk = nblk // 2
    nc = bacc.Bacc(
        "TRN2", target_bir_lowering=False, debug=False, num_devices=N_CORES,
        num_swdge_queues=NQ,
    )

    # I/O
    xt_d = nc.dram_tensor("xtab", [npad, D], BF16, kind="ExternalInput")
    w1_d = nc.dram_tensor("w1", [2, 128, D], BF16, kind="ExternalInput")
    w2_d = nc.dram_tensor("w2p", [2, 128, TCOLS], BF16, kind="ExternalInput")
    b1_d = nc.dram_tensor("b1r", [2, 128, 1], FP32, kind="ExternalInput")
    b2_d = nc.dram_tensor("b2b", [128, D], FP32, kind="ExternalInput")
    ones_d = nc.dram_tensor("ones384", [128, TCOLS], FP32, kind="ExternalInput")
    ident_d = nc.dram_tensor("ident", [128, 128], BF16, kind="ExternalInput")
    idxs_d = nc.dram_tensor("idxs", [128, 8 * st], I16, kind="ExternalInput")
    p1_d = nc.dram_tensor("p1", [128, st, 128], BF16, kind="ExternalInput")
    p2_d = nc.dram_tensor("p2", [128, st, 128], BF16, kind="ExternalInput")
    pt_d = nc.dram_tensor("pt", [128, st, 128], BF16, kind="ExternalInput")
    out_d = nc.dram_tensor("out_slice", [per, D], FP32, kind="ExternalOutput")

    # internal DRAM. table2 is chunk-major ([chunk, rank, row, col]) so each
    # chunked AllGather writes a contiguous region; gather indices (and the
    # X table) are permuted to this row order on the host.
    t2s_d = nc.dram_tensor("t2slice", [2, half, TCOLS], BF16)
    table2 = nc.dram_tensor(
        "table2", [2, N_CORES, half, TCOLS], BF16, addr_space="Shared"
    )
    t2flat = table2.reshape([npad, TCOLS])

    mu, ad, mx = (
        mybir.AluOpType.mult,
        mybir.AluOpType.add,
        mybir.AluOpType.max,
    )
    qi = [0]

    def gather(g_sb, tab, idxs_sb, b, ncols):
        """Chunked dma_gather of T[b]*128 rows into g_sb, queues round-robin."""
        tb = T[b]
        o = int(offs[b])
        t0 = 0
        for ch in range(GCH):
            t1 = min(tb, ((ch + 1) * tb + GCH - 1) // GCH)
            if t1 <= t0:
                continue
            nidx = (t1 - t0) * 128
            nc.gpsimd.dma_gather(
                g_sb[:, t0:t1, :],
                tab[:, :],
                idxs_sb[:, 8 * (o + t0): 8 * (o + t1)],
                num_idxs=nidx,
                num_idxs_reg=nidx,
                elem_size=ncols,
                single_packet=False,
                queue_num=qi[0],
            )
            qi[0] = (qi[0] + 1) % NQ
            t0 = t1

    with tile.TileContext(nc) as tc:
        with tc.tile_pool(name="const", bufs=1) as cp:
            ident_sb = cp.tile([128, 128], BF16)
            nc.sync.dma_start(ident_sb[:], ident_d[:, :])
            b2_sb = cp.tile([128, D], FP32)
            nc.sync.dma_start(b2_sb[:], b2_d[:, :])
            ones_sb = cp.tile([128, TCOLS], FP32)
            nc.sync.dma_start(ones_sb[:], ones_d[:, :])
            b1_sb = cp.tile([128, 2, 1], FP32)
            w1_sb = cp.tile([128, 2, D], BF16)
            w2_sb = cp.tile([128, 2, TCOLS], BF16)
            for k in range(2):
                nc.sync.dma_start(w1_sb[:, k, :], w1_d[k])
                nc.sync.dma_start(w2_sb[:, k, :], w2_d[k])
                nc.sync.dma_start(b1_sb[:, k, :], b1_d[k])
            idxs_sb = cp.tile([128, 8 * st], I16)
            nc.sync.dma_start(idxs_sb[:], idxs_d[:, :])
            adst_sb = cp.tile([128, nblk], BF16)

            # ---------------- layer 1: GCN + local t2 slice ----------------
            with (
                tc.tile_pool(name="g1_p", bufs=2) as g1p,
                tc.tile_pool(name="p1_p", bufs=2) as p1p,
                tc.tile_pool(name="ax_p", bufs=2) as axp,
                tc.tile_pool(name="axt_p", bufs=2) as axtp,
                tc.tile_pool(name="ht_p", bufs=2) as htp,
                tc.tile_pool(name="t2_p", bufs=2) as t2p,
                tc.psum_pool(name="pax_p", bufs=2) as pax,
                tc.psum_pool(name="ptr_p", bufs=2) as ptr,
                tc.psum_pool(name="pht_p", bufs=2) as pht,
                tc.psum_pool(name="pt2_p", bufs=2) as pt2,
            ):
                for b in range(nblk):
                    tb = T[b]
                    g1 = g1p.tile([128, tb, D], BF16, tag="g1")
                    gather(g1, xt_d, idxs_sb, b, D)
                    p1t = p1p.tile([128, tb, 128], BF16, tag="p1")
                    nc.sync.dma_start(
                        p1t[:], p1_d[:, int(offs[b]): int(offs[b]) + tb, :]
                    )
                    ps = pax.tile([128, D], FP32, tag="ax")
                    for t in range(tb):
                        nc.tensor.matmul(
                            ps[:],
                            lhsT=p1t[:, t, :],
                            rhs=g1[:, t, :],
                            start=(t == 0),
                            stop=(t == tb - 1),
                        )
                    ax = axp.tile([128, D], BF16, tag="axs")
                    nc.vector.tensor_copy(ax[:], ps[:])
                    axt = axtp.tile([128, 2, 128], BF16, tag="axt")
                    for k in range(2):
                        ptt = ptr.tile([128, 128], BF16, tag="tr")
                        nc.tensor.transpose(
                            ptt[:], ax[:, k * 128:(k + 1) * 128], ident_sb[:]
                        )
                        nc.vector.tensor_copy(axt[:, k, :], ptt[:])
                    ht = htp.tile([128, 2, 128], BF16, tag="ht")
                    for fh in range(2):
                        ph = pht.tile([128, 128], FP32, tag="hT")
                        for k in range(2):
                            nc.tensor.matmul(
                                ph[:],
                                lhsT=w1_sb[:, k, fh * 128:(fh + 1) * 128],
                                rhs=axt[:, k, :],
                                start=(k == 0),
                                stop=(k == 1),
                            )
                        nc.scalar.activation(
                            ht[:, fh, :], ph[:],
                            mybir.ActivationFunctionType.Relu,
                            bias=b1_sb[:, fh, :],
                        )
                    p2b = pt2.tile([128, TCOLS], FP32, tag="t2")
                    for fh in range(2):
                        nc.tensor.matmul(
                            p2b[:],
                            lhsT=ht[:, fh, :],
                            rhs=w2_sb[:, fh, :],
                            start=(fh == 0),
                            stop=(fh == 1),
                        )
                    nc.vector.tensor_copy(adst_sb[:, b: b + 1], p2b[:, 258:259])
                    t2row = t2p.tile([128, TCOLS], BF16, tag="t2r")
                    nc.vector.tensor_tensor(
                        t2row[:], p2b[:], ones_sb[:], op=ad
                    )
                    h, r = divmod(b, hblk)
                    nc.sync.dma_start(
                        t2s_d[h, r * 128:(r + 1) * 128, :], t2row[:]
                    )
                    if b == hblk - 1:
                        nc.gpsimd.collective_compute(
                            "AllGather",
                            mybir.AluOpType.bypass,
                            replica_groups=[list(range(N_CORES))],
                            ins=[t2s_d[0]],
                            outs=[table2[0]],
                        )
                if nblk > hblk:
                    nc.gpsimd.collective_compute(
                        "AllGather",
                        mybir.AluOpType.bypass,
                        replica_groups=[list(range(N_CORES))],
                        ins=[t2s_d[1]],
                        outs=[table2[1]],
                    )

            # ---------------- layer 2: GAT ----------------
            with (
                tc.tile_pool(name="g2_p", bufs=2) as g2p,
                tc.tile_pool(name="p2s_p", bufs=2) as p2p,
                tc.tile_pool(name="pts_p", bufs=2) as ptp,
                tc.tile_pool(name="sc_p", bufs=2) as scp,
                tc.tile_pool(name="m2_p", bufs=4) as m2p,
                tc.tile_pool(name="o_p", bufs=2) as op_,
                tc.psum_pool(name="pdv_p", bufs=2) as pdv,
                tc.psum_pool(name="pag_p", bufs=2) as pag,
            ):
                for b in range(nblk):
                    tb = T[b]
                    o = int(offs[b])
                    g2 = g2p.tile([128, tb, TCOLS], BF16, tag="g2")
                    gather(g2, t2flat, idxs_sb, b, TCOLS)
                    p2t = p2p.tile([128, tb, 128], BF16, tag="p2")
                    nc.sync.dma_start(p2t[:], p2_d[:, o: o + tb, :])
                    ptt = ptp.tile([128, tb, 128], BF16, tag="pt")
                    nc.sync.dma_start(ptt[:], pt_d[:, o: o + tb, :])
                    # per-edge adst via transposed one-hot matmuls
                    dv = pdv.tile([128, tb], FP32, tag="dv")
                    for t in range(tb):
                        nc.tensor.matmul(
                            dv[:, t: t + 1],
                            lhsT=ptt[:, t, :],
                            rhs=adst_sb[:, b: b + 1],
                            start=True,
                            stop=True,
                        )
                    # alpha = exp(leaky_relu(asrc_src + adst_dst, 0.2))
                    t0 = scp.tile([128, tb], FP32, tag="t0")
                    nc.vector.tensor_tensor(
                        t0[:], g2[:, :, 256], dv[:], op=ad
                    )
                    e = scp.tile([128, tb], FP32, tag="e")
                    nc.vector.scalar_tensor_tensor(
                        e[:], t0[:], 0.2, t0[:], op0=mu, op1=mx
                    )
                    nc.vector.tensor_scalar_min(e[:], e[:], 60.0)
                    al = scp.tile([128, tb], FP32, tag="al")
                    nc.scalar.activation(
                        al[:], e[:], mybir.ActivationFunctionType.Exp
                    )
                    ps = pag.tile([128, TCOLS], FP32, tag="agg")
                    for t in range(tb):
                        m2 = m2p.tile([128, 128], BF16, tag="m2")
                        nc.scalar.mul(m2[:], p2t[:, t, :], al[:, t: t + 1])
                        nc.tensor.matmul(
                            ps[:, 0:258],
                            lhsT=m2[:],
                            rhs=g2[:, t, 0:258],
                            start=(t == 0),
                            stop=(t == tb - 1),
                        )
                    sden = scp.tile([128, 1], FP32, tag="sden")
                    nc.vector.tensor_scalar_add(sden[:], ps[:, 257:258], 1e-16)
                    rc = scp.tile([128, 1], FP32, tag="rc")
                    nc.vector.reciprocal(rc[:], sden[:])
                    ob = op_.tile([128, D], FP32, tag="ob")
                    nc.vector.scalar_tensor_tensor(
                        ob[:], ps[:, 0:D], rc[:], b2_sb[:], op0=mu, op1=ad
                    )
                    nc.vector.tensor_scalar_max(ob[:], ob[:], 0.0)
                    nc.sync.dma_start(out_d[b * 128:(b + 1) * 128, :], ob[:])
    nc.finalize()
    return nc


# ----------------------------------------------------------------------------
# entry point
# ----------------------------------------------------------------------------

_CACHE = {}


def _get_nc(T, npad, per, nblk):
    key = (tuple(T), npad, per, nblk)
    if key not in _CACHE:
        _CACHE[key] = _build_nc(T, npad, per, nblk)
    return _CACHE[key]


def kernel(event_emb, edge_index, W1, b1, W2, att_src, att_dst, b2,
           _want_results=False, _trace=False):
    shared, per_core, T, n, npad, per, nblk = _prep(
        event_emb, edge_index, W1, b1, W2, att_src, att_dst, b2
    )
    nc = _get_nc(T, npad, per, nblk)
    in_maps = [{**shared, **per_core[c]} for c in range(N_CORES)]
    res = run_bass_kernel_spmd(
        nc, in_maps, core_ids=list(range(N_CORES)), trace=_trace
    )
    out = np.concatenate(
        [res.results[c]["out_slice"] for c in range(N_CORES)], axis=0
    )[:n]
    if _want_results:
        return out, res
    return out


# revision 10
# speedup vs baseline: 2.6782x; 1.2273x over previous
"""Trainium2 Bass kernel for EventDiffusion GNN (GCNConv + GATConv, 2 layers).

Sharding: nodes partitioned into 8 contiguous ranges (one per NeuronCore).
Each core aggregates messages for its destination-node range from replicated
tables (graph/data parallel per the sharding hint); the layer-1 output table
is exchanged with an AllGather so every core can gather arbitrary source rows
for layer 2.

Dataflow per core (all feature data bf16, PSUM accumulation fp32):
  L1 (GCN):  gather X[src] rows per edge (HW dma_gather over 4 SWDGE queues),
             scatter-sum by dst-slot via host-precomputed one-hot matrices
             P1[e, j] = coeff_e * (dslot_e == j) streamed bf16:
             psum += P1^T @ gathered.  Then h = relu(AX @ W1 + b1) via PE
             transposes + matmuls, and the layer-2 row table
             t2[d, :] = [h@W2 | h.v1 | 1 | h.v2] is built locally per block.
  comm:      AllGather of the local t2 slice in 2 chunks; chunk 0 is issued
             mid-L1 so its ring transfer hides under L1 gathers.
  L2 (GAT):  two passes split by source chunk so pass-A gathers only need
             collective chunk 0 and start while chunk 1 is still in flight
             (the SWDGE gather pipe never drains).  Per-edge dst values via
             psum_dv = PT^T @ adst (PT = transposed one-hot); alpha = exp of
             unshifted logits (logits are O(1), no overflow risk; the softmax
             shift cancels exactly in numerator/denominator); one-hot scaled
             by alpha on the Scalar engine; psum += (P2*alpha)^T @ gathered.
             The denominator rides along as the all-ones table column.
"""

import numpy as np
import ml_dtypes

import concourse.bass as bass
import concourse.bacc as bacc
import concourse.mybir as mybir
import concourse.tile as tile
from concourse.bass_utils import run_bass_kernel_spmd

FP32 = mybir.dt.float32
BF16 = mybir.dt.bfloat16
I16 = mybir.dt.int16
NPBF16 = ml_dtypes.bfloat16

N_CORES = 8
D = 256
TCOLS = 384  # t2 row: [xw2(0:256) | asrc(256) | one(257) | adst(258) | pad)
NQ = 4       # SWDGE queues
GCH = 4      # gather chunks per block (round-robin over queues)


def _pad_nodes(n):
    return -(-n // (128 * N_CORES)) * (128 * N_CORES)


def _wrap16(idx):
    s = idx.astype(np.int16).reshape(-1, 16).T  # [16, L/16]
    return np.tile(s, (8, 1))  # [128, L/16]


# ----------------------------------------------------------------------------
# host-side preprocessing (graph structure only: indices + one-hot scatters)
# ----------------------------------------------------------------------------

def _tiles(src_l, dslot_l, coeff_l, Ts, with_coeff):
    """Pack per-(block) edge lists into 128-row tiles.

    Returns idxs [128, 8*st], P [128, st, 128] (one-hot or coeff-one-hot) and
    PT [128, st, 128] (transposed one-hot) for the concatenated tile list.
    """
    st = sum(Ts)
    idxs = np.zeros((128, 8 * st), np.int16)
    P = np.zeros((128, st, 128), NPBF16)
    PT = np.zeros((128, st, 128), NPBF16)
    off = 0
    for s, j, co, T in zip(src_l, dslot_l, coeff_l, Ts):
        m = len(s)
        L = T * 128
        e = np.arange(m)
        t = off + e // 128
        p = e % 128
        if with_coeff:
            P[p, t, j] = co.astype(NPBF16)
        else:
            P[p, t, j] = NPBF16(1.0)
        PT[j, t, p] = NPBF16(1.0)
        sfull = np.zeros(L, np.int64)
        sfull[:m] = s
        idxs[:, 8 * off: 8 * (off + T)] = _wrap16(sfull)
        off += T
    return idxs, P, PT


def _prep(event_emb, edge_index, W1, b1, W2, att_src, att_dst, b2):
    X = np.ascontiguousarray(np.asarray(event_emb, np.float32))
    n = X.shape[0]
    npad = _pad_nodes(n)
    per = npad // N_CORES
    nblk = per // 128
    half = per // 2

    ei = np.asarray(edge_index, np.int64)
    src = np.concatenate([ei[0], np.arange(n, dtype=np.int64)])
    dst = np.concatenate([ei[1], np.arange(n, dtype=np.int64)])
    deg = np.bincount(dst, minlength=n).astype(np.float32)
    dinv = np.where(deg > 0, 1.0 / np.sqrt(deg), 0.0).astype(np.float32)
    coeff = (dinv[src] * dinv[dst]).astype(np.float32)

    # chunk-major row permutation matching the on-device table2 layout
    # ([chunk, rank, row]): global row g -> h*(8*half) + c*half + rr
    def _perm(g):
        c, r = g // per, g % per
        return (r // half) * (N_CORES * half) + c * half + (r % half)

    key = (dst // per) * nblk + (dst % per) // 128
    order = np.argsort(key, kind="stable")
    src, dst, coeff, key = src[order], dst[order], coeff[order], key[order]
    srcp = _perm(src)
    schunk = (src % per) // half  # source collective chunk (0 or 1)
    bounds = np.searchsorted(key, np.arange(N_CORES * nblk + 1))

    # per-(core, block) edge lists: full (L1) and split by source chunk (L2)
    s1, j1, c1 = [[[] for _ in range(nblk)] for _ in range(3)]
    sA, jA, sB, jB = [[[] for _ in range(nblk)] for _ in range(4)]
    for c in range(N_CORES):
        for b in range(nblk):
            lo, hi = bounds[c * nblk + b], bounds[c * nblk + b + 1]
            j = (dst[lo:hi] - (c * per + b * 128)).astype(np.int64)
            s1[b].append(srcp[lo:hi])
            j1[b].append(j)
            c1[b].append(coeff[lo:hi])
            m = schunk[lo:hi] == 0
            sA[b].append(srcp[lo:hi][m])
            jA[b].append(j[m])
            sB[b].append(srcp[lo:hi][~m])
            jB[b].append(j[~m])

    def tmax(ll):
        return [max(1, int(-(-max(len(x) for x in ll[b]) // 128)))
                for b in range(nblk)]

    T1, TA, TB = tmax(s1), tmax(sA), tmax(sB)

    per_core = []
    zco = [None] * nblk
    for c in range(N_CORES):
        idxs1, p1, _ = _tiles(
            [s1[b][c] for b in range(nblk)], [j1[b][c] for b in range(nblk)],
            [c1[b][c] for b in range(nblk)], T1, True,
        )
        sl = [sA[b][c] for b in range(nblk)] + [sB[b][c] for b in range(nblk)]
        jl = [jA[b][c] for b in range(nblk)] + [jB[b][c] for b in range(nblk)]
        idxs2, p2, pt = _tiles(sl, jl, zco + zco, TA + TB, False)
        per_core.append(dict(idxs1=idxs1, p1=p1, idxs2=idxs2, p2=p2, pt=pt))

    W1 = np.asarray(W1, np.float32)
    W2 = np.asarray(W2, np.float32)
    v1 = W2 @ np.asarray(att_src, np.float32)
    v2 = W2 @ np.asarray(att_dst, np.float32)

    Xp = np.zeros((npad, D), NPBF16)
    Xp[_perm(np.arange(n))] = X.astype(NPBF16)

    W2p = np.zeros((D, TCOLS), np.float32)
    W2p[:, :D] = W2
    W2p[:, 256] = v1
    W2p[:, 258] = v2

    ones384 = np.zeros((128, TCOLS), np.float32)
    ones384[:, 257] = 1.0

    shared = dict(
        xtab=Xp,
        w1=np.ascontiguousarray(W1.reshape(2, 128, D).astype(NPBF16)),
        w2p=np.ascontiguousarray(W2p.reshape(2, 128, TCOLS).astype(NPBF16)),
        b1r=np.ascontiguousarray(
            np.asarray(b1, np.float32).reshape(2, 128, 1)
        ),
        b2b=np.ascontiguousarray(
            np.tile(np.asarray(b2, np.float32)[None, :], (128, 1))
        ),
        ones384=ones384,
        ident=np.eye(128, dtype=NPBF16),
    )
    return shared, per_core, (T1, TA, TB), n, npad, per, nblk


# ----------------------------------------------------------------------------
# device program
# ----------------------------------------------------------------------------

def _build_nc(T1, TA, TB, npad, per, nblk):
    st1 = sum(T1)
    st2 = sum(TA) + sum(TB)
    offs1 = np.concatenate([[0], np.cumsum(T1)]).astype(np.int64)
    offs2 = np.concatenate([[0], np.cumsum(TA + TB)]).astype(np.int64)
    half = per // 2
    hblk = nblk // 2
    nc = bacc.Bacc(
        "TRN2", target_bir_lowering=False, debug=False, num_devices=N_CORES,
        num_swdge_queues=NQ, dynamic_dma_scratch_size=32768,
    )

    # I/O
    xt_d = nc.dram_tensor("xtab", [npad, D], BF16, kind="ExternalInput")
    w1_d = nc.dram_tensor("w1", [2, 128, D], BF16, kind="ExternalInput")
    w2_d = nc.dram_tensor("w2p", [2, 128, TCOLS], BF16, kind="ExternalInput")
    b1_d = nc.dram_tensor("b1r", [2, 128, 1], FP32, kind="ExternalInput")
    b2_d = nc.dram_tensor("b2b", [128, D], FP32, kind="ExternalInput")
    ones_d = nc.dram_tensor("ones384", [128, TCOLS], FP32, kind="ExternalInput")
    ident_d = nc.dram_tensor("ident", [128, 128], BF16, kind="ExternalInput")
    idxs1_d = nc.dram_tensor("idxs1", [128, 8 * st1], I16, kind="ExternalInput")
    p1_d = nc.dram_tensor("p1", [128, st1, 128], BF16, kind="ExternalInput")
    idxs2_d = nc.dram_tensor("idxs2", [128, 8 * st2], I16, kind="ExternalInput")
    p2_d = nc.dram_tensor("p2", [128, st2, 128], BF16, kind="ExternalInput")
    pt_d = nc.dram_tensor("pt", [128, st2, 128], BF16, kind="ExternalInput")
    out_d = nc.dram_tensor("out_slice", [per, D], FP32, kind="ExternalOutput")

    # internal DRAM. table2 is chunk-major ([chunk, rank, row, col]) so each
    # chunked AllGather writes a contiguous region; gather indices (and the
    # X table) are permuted to this row order on the host.
    t2s_d = nc.dram_tensor("t2slice", [2, half, TCOLS], BF16)
    table2 = nc.dram_tensor(
        "table2", [2, N_CORES, half, TCOLS], BF16, addr_space="Shared"
    )
    t2flat = table2.reshape([npad, TCOLS])

    mu, ad, mx = (
        mybir.AluOpType.mult,
        mybir.AluOpType.add,
        mybir.AluOpType.max,
    )
    qi = [0]

    def gather(g_sb, tab, idxs_sb, tb, o, ncols):
        """Chunked dma_gather of tb*128 rows into g_sb, queues round-robin."""
        t0 = 0
        for ch in range(GCH):
            t1 = min(tb, ((ch + 1) * tb + GCH - 1) // GCH)
            if t1 <= t0:
                continue
            nidx = (t1 - t0) * 128
            nc.gpsimd.dma_gather(
                g_sb[:, t0:t1, :],
                tab[:, :],
                idxs_sb[:, 8 * (o + t0): 8 * (o + t1)],
                num_idxs=nidx,
                num_idxs_reg=nidx,
                elem_size=ncols,
                single_packet=False,
                queue_num=qi[0],
            )
            qi[0] = (qi[0] + 1) % NQ
            t0 = t1

    def allgather(ch):
        nc.gpsimd.collective_compute(
            "AllGather",
            mybir.AluOpType.bypass,
            replica_groups=[list(range(N_CORES))],
            ins=[t2s_d[ch]],
            outs=[table2[ch]],
        )

    with tile.TileContext(nc) as tc:
        with tc.tile_pool(name="const", bufs=1) as cp:
            ident_sb = cp.tile([128, 128], BF16)
            nc.sync.dma_start(ident_sb[:], ident_d[:, :])
            b2_sb = cp.tile([128, D], FP32)
            nc.sync.dma_start(b2_sb[:], b2_d[:, :])
            ones_sb = cp.tile([128, TCOLS], FP32)
            nc.sync.dma_start(ones_sb[:], ones_d[:, :])
            b1_sb = cp.tile([128, 2, 1], FP32)
            w1_sb = cp.tile([128, 2, D], BF16)
            w2_sb = cp.tile([128, 2, TCOLS], BF16)
            for k in range(2):
                nc.sync.dma_start(w1_sb[:, k, :], w1_d[k])
                nc.sync.dma_start(w2_sb[:, k, :], w2_d[k])
                nc.sync.dma_start(b1_sb[:, k, :], b1_d[k])
            idxs1_sb = cp.tile([128, 8 * st1], I16)
            nc.sync.dma_start(idxs1_sb[:], idxs1_d[:, :])
            idxs2_sb = cp.tile([128, 8 * st2], I16)
            nc.sync.dma_start(idxs2_sb[:], idxs2_d[:, :])
            adst_sb = cp.tile([128, nblk], BF16)
            partA = cp.tile([128, nblk, 258], FP32)

            # ---------------- layer 1: GCN + local t2 slice ----------------
            with (
                tc.tile_pool(name="g1_p", bufs=3) as g1p,
                tc.tile_pool(name="p1_p", bufs=3) as p1p,
                tc.tile_pool(name="ax_p", bufs=2) as axp,
                tc.tile_pool(name="axt_p", bufs=2) as axtp,
                tc.tile_pool(name="ht_p", bufs=2) as htp,
                tc.tile_pool(name="t2_p", bufs=2) as t2p,
                tc.psum_pool(name="pax_p", bufs=2) as pax,
                tc.psum_pool(name="ptr_p", bufs=2) as ptr,
                tc.psum_pool(name="pht_p", bufs=2) as pht,
                tc.psum_pool(name="pt2_p", bufs=2) as pt2,
            ):
                for b in range(nblk):
                    tb = T1[b]
                    o = int(offs1[b])
                    g1 = g1p.tile([128, tb, D], BF16, tag="g1")
                    gather(g1, xt_d, idxs1_sb, tb, o, D)
                    if b == hblk + 1:
                        allgather(0)
                    p1t = p1p.tile([128, tb, 128], BF16, tag="p1")
                    nc.sync.dma_start(p1t[:], p1_d[:, o: o + tb, :])
                    ps = pax.tile([128, D], FP32, tag="ax")
                    for t in range(tb):
                        nc.tensor.matmul(
                            ps[:],
                            lhsT=p1t[:, t, :],
                            rhs=g1[:, t, :],
                            start=(t == 0),
                            stop=(t == tb - 1),
                        )
                    ax = axp.tile([128, D], BF16, tag="axs")
                    nc.vector.tensor_copy(ax[:], ps[:])
                    axt = axtp.tile([128, 2, 128], BF16, tag="axt")
                    for k in range(2):
                        ptt = ptr.tile([128, 128], BF16, tag="tr")
                        nc.tensor.transpose(
                            ptt[:], ax[:, k * 128:(k + 1) * 128], ident_sb[:]
                        )
                        nc.vector.tensor_copy(axt[:, k, :], ptt[:])
                    ht = htp.tile([128, 2, 128], BF16, tag="ht")
                    for fh in range(2):
                        ph = pht.tile([128, 128], FP32, tag="hT")
                        for k in range(2):
                            nc.tensor.matmul(
                                ph[:],
                                lhsT=w1_sb[:, k, fh * 128:(fh + 1) * 128],
                                rhs=axt[:, k, :],
                                start=(k == 0),
                                stop=(k == 1),
                            )
                        nc.scalar.activation(
                            ht[:, fh, :], ph[:],
                            mybir.ActivationFunctionType.Relu,
                            bias=b1_sb[:, fh, :],
                        )
                    p2b = pt2.tile([128, TCOLS], FP32, tag="t2")
                    for fh in range(2):
                        nc.tensor.matmul(
                            p2b[:],
                            lhsT=ht[:, fh, :],
                            rhs=w2_sb[:, fh, :],
                            start=(fh == 0),
                            stop=(fh == 1),
                        )
                    nc.vector.tensor_copy(adst_sb[:, b: b + 1], p2b[:, 258:259])
                    t2row = t2p.tile([128, TCOLS], BF16, tag="t2r")
                    nc.vector.tensor_tensor(
                        t2row[:], p2b[:], ones_sb[:], op=ad
                    )
                    h, r = divmod(b, hblk)
                    nc.sync.dma_start(
                        t2s_d[h, r * 128:(r + 1) * 128, :], t2row[:]
                    )
                if nblk > hblk:
                    allgather(1)

            # ---------------- layer 2: GAT (two passes by src chunk) -------
            with (
                tc.tile_pool(name="g2_p", bufs=3) as g2p,
                tc.tile_pool(name="p2s_p", bufs=3) as p2p,
                tc.tile_pool(name="pts_p", bufs=3) as ptp,
                tc.tile_pool(name="sc_p", bufs=3) as scp,
                tc.tile_pool(name="m2_p", bufs=4) as m2p,
                tc.tile_pool(name="o_p", bufs=2) as op_,
                tc.psum_pool(name="pdv_p", bufs=2) as pdv,
                tc.psum_pool(name="pag_p", bufs=2) as pag,
            ):
                for ps_ in range(2):
                    Tp = TA if ps_ == 0 else TB
                    for b in range(nblk):
                        tb = Tp[b]
                        o = int(offs2[ps_ * nblk + b])
                        g2 = g2p.tile([128, tb, TCOLS], BF16, tag="g2")
                        gather(g2, t2flat, idxs2_sb, tb, o, TCOLS)
                        p2t = p2p.tile([128, tb, 128], BF16, tag="p2")
                        nc.sync.dma_start(p2t[:], p2_d[:, o: o + tb, :])
                        ptt = ptp.tile([128, tb, 128], BF16, tag="pt")
                        nc.sync.dma_start(ptt[:], pt_d[:, o: o + tb, :])
                        # per-edge adst via transposed one-hot matmuls
                        dv = pdv.tile([128, tb], FP32, tag="dv")
                        for t in range(tb):
                            nc.tensor.matmul(
                                dv[:, t: t + 1],
                                lhsT=ptt[:, t, :],
                                rhs=adst_sb[:, b: b + 1],
                                start=True,
                                stop=True,
                            )
                        # alpha = exp(leaky_relu(asrc_src + adst_dst, 0.2))
                        t0 = scp.tile([128, tb], FP32, tag="t0")
                        nc.vector.tensor_tensor(
                            t0[:], g2[:, :, 256], dv[:], op=ad
                        )
                        e = scp.tile([128, tb], FP32, tag="e")
                        nc.vector.scalar_tensor_tensor(
                            e[:], t0[:], 0.2, t0[:], op0=mu, op1=mx
                        )
                        nc.vector.tensor_scalar_min(e[:], e[:], 60.0)
                        al = scp.tile([128, tb], FP32, tag="al")
                        nc.scalar.activation(
                            al[:], e[:], mybir.ActivationFunctionType.Exp
                        )
                        ps = pag.tile([128, 258], FP32, tag="agg")
                        for t in range(tb):
                            m2 = m2p.tile([128, 128], BF16, tag="m2")
                            nc.scalar.mul(m2[:], p2t[:, t, :], al[:, t: t + 1])
                            nc.tensor.matmul(
                                ps[:],
                                lhsT=m2[:],
                                rhs=g2[:, t, 0:258],
                                start=(t == 0),
                                stop=(t == tb - 1),
                            )
                        if ps_ == 0:
                            nc.vector.tensor_copy(partA[:, b, :], ps[:])
                            continue
                        num = op_.tile([128, 258], FP32, tag="num")
                        nc.vector.tensor_tensor(
                            num[:], ps[:], partA[:, b, :], op=ad
                        )
                        sden = scp.tile([128, 1], FP32, tag="sden")
                        nc.vector.tensor_scalar_add(
                            sden[:], num[:, 257:258], 1e-16
                        )
                        rc = scp.tile([128, 1], FP32, tag="rc")
                        nc.vector.reciprocal(rc[:], sden[:])
                        ob = op_.tile([128, D], FP32, tag="ob")
                        nc.vector.scalar_tensor_tensor(
                            ob[:], num[:, 0:D], rc[:], b2_sb[:], op0=mu, op1=ad
                        )
                        nc.vector.tensor_scalar_max(ob[:], ob[:], 0.0)
                        nc.sync.dma_start(
                            out_d[b * 128:(b + 1) * 128, :], ob[:]
                        )
    nc.finalize()
    return nc


# ----------------------------------------------------------------------------
# entry point
# ----------------------------------------------------------------------------

_CACHE = {}


def _get_nc(T1, TA, TB, npad, per, nblk):
    key = (tuple(T1), tuple(TA), tuple(TB), npad, per, nblk)
    if key not in _CACHE:
        _CACHE[key] = _build_nc(T1, TA, TB, npad, per, nblk)
    return _CACHE[key]


def kernel(event_emb, edge_index, W1, b1, W2, att_src, att_dst, b2,
           _want_results=False, _trace=False):
    shared, per_core, (T1, TA, TB), n, npad, per, nblk = _prep(
        event_emb, edge_index, W1, b1, W2, att_src, att_dst, b2
    )
    nc = _get_nc(T1, TA, TB, npad, per, nblk)
    in_maps = [{**shared, **per_core[c]} for c in range(N_CORES)]
    res = run_bass_kernel_spmd(
        nc, in_maps, core_ids=list(range(N_CORES)), trace=_trace
    )
    out = np.concatenate(
        [res.results[c]["out_slice"] for c in range(N_CORES)], axis=0
    )[:n]
    if _want_results:
        return out, res
    return out


# revision 13
# speedup vs baseline: 2.7736x; 1.0356x over previous
"""Trainium2 Bass kernel for EventDiffusion GNN (GCNConv + GATConv, 2 layers).

Sharding: nodes partitioned into 8 contiguous ranges (one per NeuronCore).
Each core aggregates messages for its destination-node range from replicated
tables (graph/data parallel per the sharding hint); the layer-1 output table
is exchanged with an AllGather so every core can gather arbitrary source rows
for layer 2.

Dataflow per core (all feature data bf16, PSUM accumulation fp32):
  L1 (GCN):  gather X[src] rows per edge (HW dma_gather over 4 SWDGE queues),
             scatter-sum by dst-slot via host-precomputed one-hot matrices
             P1[e, j] = coeff_e * (dslot_e == j) streamed bf16:
             psum += P1^T @ gathered.  Then h = relu(AX @ W1 + b1) via PE
             transposes + matmuls, and the layer-2 row table
             t2[d, :] = [h@W2 | h.v1 | 1 | h.v2] is built locally per block.
  comm:      AllGather of the local t2 slice in 2 chunks; chunk 0 is issued
             mid-L1 so its ring transfer hides under L1 gathers.
  L2 (GAT):  two passes split by source chunk so pass-A gathers only need
             collective chunk 0 and start while chunk 1 is still in flight
             (the SWDGE gather pipe never drains).  Per-edge dst values via
             psum_dv = PT^T @ adst (PT = transposed one-hot); alpha = exp of
             unshifted logits (logits are O(1), no overflow risk; the softmax
             shift cancels exactly in numerator/denominator); one-hot scaled
             by alpha on the Scalar engine; psum += (P2*alpha)^T @ gathered.
             The denominator rides along as the all-ones table column.
"""

import numpy as np
import ml_dtypes

import concourse.bass as bass
import concourse.bacc as bacc
import concourse.mybir as mybir
import concourse.tile as tile
from concourse.bass_utils import run_bass_kernel_spmd

FP32 = mybir.dt.float32
BF16 = mybir.dt.bfloat16
I16 = mybir.dt.int16
NPBF16 = ml_dtypes.bfloat16

N_CORES = 8
D = 256
TCOLS = 384  # t2 row: [xw2(0:256) | asrc(256) | one(257) | adst(258) | pad)
NQ = 4       # SWDGE queues
GCH = 4      # gather chunks per block (round-robin over queues)


def _pad_nodes(n):
    return -(-n // (128 * N_CORES)) * (128 * N_CORES)


def _wrap16(idx):
    s = idx.astype(np.int16).reshape(-1, 16).T  # [16, L/16]
    return np.tile(s, (8, 1))  # [128, L/16]


# ----------------------------------------------------------------------------
# host-side preprocessing (graph structure only: indices + one-hot scatters)
# ----------------------------------------------------------------------------

def _tiles(src_l, dslot_l, coeff_l, Ts, with_coeff):
    """Pack per-(block) edge lists into 128-row tiles.

    Returns idxs [128, 8*st], P [128, st, 128] (one-hot or coeff-one-hot) and
    PT [128, st, 128] (transposed one-hot) for the concatenated tile list.
    """
    st = sum(Ts)
    idxs = np.zeros((128, 8 * st), np.int16)
    P = np.zeros((128, st, 128), NPBF16)
    PT = np.zeros((128, st, 128), NPBF16)
    off = 0
    for s, j, co, T in zip(src_l, dslot_l, coeff_l, Ts):
        m = len(s)
        L = T * 128
        e = np.arange(m)
        t = off + e // 128
        p = e % 128
        if with_coeff:
            P[p, t, j] = co.astype(NPBF16)
        else:
            P[p, t, j] = NPBF16(1.0)
        PT[j, t, p] = NPBF16(1.0)
        sfull = np.zeros(L, np.int64)
        sfull[:m] = s
        idxs[:, 8 * off: 8 * (off + T)] = _wrap16(sfull)
        off += T
    return idxs, P, PT


def _prep(event_emb, edge_index, W1, b1, W2, att_src, att_dst, b2):
    X = np.ascontiguousarray(np.asarray(event_emb, np.float32))
    n = X.shape[0]
    npad = _pad_nodes(n)
    per = npad // N_CORES
    nblk = per // 128
    half = per // 2

    ei = np.asarray(edge_index, np.int64)
    src = np.concatenate([ei[0], np.arange(n, dtype=np.int64)])
    dst = np.concatenate([ei[1], np.arange(n, dtype=np.int64)])
    deg = np.bincount(dst, minlength=n).astype(np.float32)
    dinv = np.where(deg > 0, 1.0 / np.sqrt(deg), 0.0).astype(np.float32)
    coeff = (dinv[src] * dinv[dst]).astype(np.float32)

    # chunk-major row permutation matching the on-device table2 layout
    # ([chunk, rank, row]): global row g -> h*(8*half) + c*half + rr
    def _perm(g):
        c, r = g // per, g % per
        return (r // half) * (N_CORES * half) + c * half + (r % half)

    key = (dst // per) * nblk + (dst % per) // 128
    order = np.argsort(key, kind="stable")
    src, dst, coeff, key = src[order], dst[order], coeff[order], key[order]
    srcp = _perm(src)
    schunk = (src % per) // half  # source collective chunk (0 or 1)
    bounds = np.searchsorted(key, np.arange(N_CORES * nblk + 1))

    # per-(core, block) edge lists: full (L1) and split by source chunk (L2)
    s1, j1, c1 = [[[] for _ in range(nblk)] for _ in range(3)]
    sA, jA, sB, jB = [[[] for _ in range(nblk)] for _ in range(4)]
    for c in range(N_CORES):
        for b in range(nblk):
            lo, hi = bounds[c * nblk + b], bounds[c * nblk + b + 1]
            j = (dst[lo:hi] - (c * per + b * 128)).astype(np.int64)
            s1[b].append(srcp[lo:hi])
            j1[b].append(j)
            c1[b].append(coeff[lo:hi])
            m = schunk[lo:hi] == 0
            sA[b].append(srcp[lo:hi][m])
            jA[b].append(j[m])
            # pass-B indices are rebased into the chunk-1 sub-table so the
            # gather's input AP (and thus its dependency) covers only chunk 1
            sB[b].append(srcp[lo:hi][~m] - N_CORES * half)
            jB[b].append(j[~m])

    def tmax(ll):
        return [max(1, int(-(-max(len(x) for x in ll[b]) // 128)))
                for b in range(nblk)]

    T1, TA, TB = tmax(s1), tmax(sA), tmax(sB)

    per_core = []
    zco = [None] * nblk
    for c in range(N_CORES):
        idxs1, p1, _ = _tiles(
            [s1[b][c] for b in range(nblk)], [j1[b][c] for b in range(nblk)],
            [c1[b][c] for b in range(nblk)], T1, True,
        )
        sl = [sA[b][c] for b in range(nblk)] + [sB[b][c] for b in range(nblk)]
        jl = [jA[b][c] for b in range(nblk)] + [jB[b][c] for b in range(nblk)]
        idxs2, p2, pt = _tiles(sl, jl, zco + zco, TA + TB, False)
        per_core.append(dict(idxs1=idxs1, p1=p1, idxs2=idxs2, p2=p2, pt=pt))

    W1 = np.asarray(W1, np.float32)
    W2 = np.asarray(W2, np.float32)
    v1 = W2 @ np.asarray(att_src, np.float32)
    v2 = W2 @ np.asarray(att_dst, np.float32)

    Xp = np.zeros((npad, D), NPBF16)
    Xp[_perm(np.arange(n))] = X.astype(NPBF16)

    W2p = np.zeros((D, TCOLS), np.float32)
    W2p[:, :D] = W2
    W2p[:, 256] = v1
    W2p[:, 258] = v2

    ones384 = np.zeros((128, TCOLS), np.float32)
    ones384[:, 257] = 1.0

    shared = dict(
        xtab=Xp,
        w1=np.ascontiguousarray(W1.reshape(2, 128, D).astype(NPBF16)),
        w2p=np.ascontiguousarray(W2p.reshape(2, 128, TCOLS).astype(NPBF16)),
        b1r=np.ascontiguousarray(
            np.asarray(b1, np.float32).reshape(2, 128, 1)
        ),
        b2b=np.ascontiguousarray(
            np.tile(np.asarray(b2, np.float32)[None, :], (128, 1))
        ),
        ones384=ones384,
        ident=np.eye(128, dtype=NPBF16),
    )
    return shared, per_core, (T1, TA, TB), n, npad, per, nblk


# ----------------------------------------------------------------------------
# device program
# ----------------------------------------------------------------------------

def _build_nc(T1, TA, TB, npad, per, nblk):
    st1 = sum(T1)
    st2 = sum(TA) + sum(TB)
    offs1 = np.concatenate([[0], np.cumsum(T1)]).astype(np.int64)
    offs2 = np.concatenate([[0], np.cumsum(TA + TB)]).astype(np.int64)
    half = per // 2
    hblk = nblk // 2
    nc = bacc.Bacc(
        "TRN2", target_bir_lowering=False, debug=False, num_devices=N_CORES,
        num_swdge_queues=NQ, dynamic_dma_scratch_size=32768,
    )

    # I/O
    xt_d = nc.dram_tensor("xtab", [npad, D], BF16, kind="ExternalInput")
    w1_d = nc.dram_tensor("w1", [2, 128, D], BF16, kind="ExternalInput")
    w2_d = nc.dram_tensor("w2p", [2, 128, TCOLS], BF16, kind="ExternalInput")
    b1_d = nc.dram_tensor("b1r", [2, 128, 1], FP32, kind="ExternalInput")
    b2_d = nc.dram_tensor("b2b", [128, D], FP32, kind="ExternalInput")
    ones_d = nc.dram_tensor("ones384", [128, TCOLS], FP32, kind="ExternalInput")
    ident_d = nc.dram_tensor("ident", [128, 128], BF16, kind="ExternalInput")
    idxs1_d = nc.dram_tensor("idxs1", [128, 8 * st1], I16, kind="ExternalInput")
    p1_d = nc.dram_tensor("p1", [128, st1, 128], BF16, kind="ExternalInput")
    idxs2_d = nc.dram_tensor("idxs2", [128, 8 * st2], I16, kind="ExternalInput")
    p2_d = nc.dram_tensor("p2", [128, st2, 128], BF16, kind="ExternalInput")
    pt_d = nc.dram_tensor("pt", [128, st2, 128], BF16, kind="ExternalInput")
    out_d = nc.dram_tensor("out_slice", [per, D], FP32, kind="ExternalOutput")

    # internal DRAM. table2 is chunk-major ([chunk, rank, row, col]) so each
    # chunked AllGather writes a contiguous region; gather indices (and the
    # X table) are permuted to this row order on the host.
    t2s_d = nc.dram_tensor("t2slice", [2, half, TCOLS], BF16)
    table2 = nc.dram_tensor(
        "table2", [2, N_CORES, half, TCOLS], BF16, addr_space="Shared"
    )
    t2chunk = table2.reshape([2, N_CORES * half, TCOLS])

    mu, ad, mx = (
        mybir.AluOpType.mult,
        mybir.AluOpType.add,
        mybir.AluOpType.max,
    )
    qi = [0]

    def gather(g_sb, tab, idxs_sb, tb, o, ncols):
        """Chunked dma_gather of tb*128 rows into g_sb, queues round-robin."""
        t0 = 0
        for ch in range(GCH):
            t1 = min(tb, ((ch + 1) * tb + GCH - 1) // GCH)
            if t1 <= t0:
                continue
            nidx = (t1 - t0) * 128
            nc.gpsimd.dma_gather(
                g_sb[:, t0:t1, :],
                tab[:, :],
                idxs_sb[:, 8 * (o + t0): 8 * (o + t1)],
                num_idxs=nidx,
                num_idxs_reg=nidx,
                elem_size=ncols,
                single_packet=False,
                queue_num=qi[0],
            )
            qi[0] = (qi[0] + 1) % NQ
            t0 = t1

    def allgather(ch):
        nc.gpsimd.collective_compute(
            "AllGather",
            mybir.AluOpType.bypass,
            replica_groups=[list(range(N_CORES))],
            ins=[t2s_d[ch]],
            outs=[table2[ch]],
        )

    with tile.TileContext(nc) as tc:
        with tc.tile_pool(name="const", bufs=1) as cp:
            ident_sb = cp.tile([128, 128], BF16)
            nc.sync.dma_start(ident_sb[:], ident_d[:, :])
            b2_sb = cp.tile([128, D], FP32)
            nc.sync.dma_start(b2_sb[:], b2_d[:, :])
            ones_sb = cp.tile([128, TCOLS], FP32)
            nc.sync.dma_start(ones_sb[:], ones_d[:, :])
            b1_sb = cp.tile([128, 2, 1], FP32)
            w1_sb = cp.tile([128, 2, D], BF16)
            w2_sb = cp.tile([128, 2, TCOLS], BF16)
            for k in range(2):
                nc.sync.dma_start(w1_sb[:, k, :], w1_d[k])
                nc.sync.dma_start(w2_sb[:, k, :], w2_d[k])
                nc.sync.dma_start(b1_sb[:, k, :], b1_d[k])
            idxs1_sb = cp.tile([128, 8 * st1], I16)
            nc.sync.dma_start(idxs1_sb[:], idxs1_d[:, :])
            idxs2_sb = cp.tile([128, 8 * st2], I16)
            nc.sync.dma_start(idxs2_sb[:], idxs2_d[:, :])
            adst_sb = cp.tile([128, nblk], BF16)
            partA = cp.tile([128, nblk, 258], FP32)

            # ---------------- layer 1: GCN + local t2 slice ----------------
            with (
                tc.tile_pool(name="g1_p", bufs=3) as g1p,
                tc.tile_pool(name="p1_p", bufs=3) as p1p,
                tc.tile_pool(name="ax_p", bufs=2) as axp,
                tc.tile_pool(name="axt_p", bufs=2) as axtp,
                tc.tile_pool(name="ht_p", bufs=2) as htp,
                tc.tile_pool(name="t2_p", bufs=2) as t2p,
                tc.psum_pool(name="pax_p", bufs=2) as pax,
                tc.psum_pool(name="ptr_p", bufs=2) as ptr,
                tc.psum_pool(name="pht_p", bufs=2) as pht,
                tc.psum_pool(name="pt2_p", bufs=2) as pt2,
            ):
                for b in range(nblk):
                    tb = T1[b]
                    o = int(offs1[b])
                    g1 = g1p.tile([128, tb, D], BF16, tag="g1")
                    gather(g1, xt_d, idxs1_sb, tb, o, D)
                    if b == hblk + 1:
                        allgather(0)
                    p1t = p1p.tile([128, tb, 128], BF16, tag="p1")
                    nc.sync.dma_start(p1t[:], p1_d[:, o: o + tb, :])
                    ps = pax.tile([128, D], FP32, tag="ax")
                    for t in range(tb):
                        nc.tensor.matmul(
                            ps[:],
                            lhsT=p1t[:, t, :],
                            rhs=g1[:, t, :],
                            start=(t == 0),
                            stop=(t == tb - 1),
                        )
                    ax = axp.tile([128, D], BF16, tag="axs")
                    nc.vector.tensor_copy(ax[:], ps[:])
                    axt = axtp.tile([128, 2, 128], BF16, tag="axt")
                    for k in range(2):
                        ptt = ptr.tile([128, 128], BF16, tag="tr")
                        nc.tensor.transpose(
                            ptt[:], ax[:, k * 128:(k + 1) * 128], ident_sb[:]
                        )
                        nc.vector.tensor_copy(axt[:, k, :], ptt[:])
                    ht = htp.tile([128, 2, 128], BF16, tag="ht")
                    for fh in range(2):
                        ph = pht.tile([128, 128], FP32, tag="hT")
                        for k in range(2):
                            nc.tensor.matmul(
                                ph[:],
                                lhsT=w1_sb[:, k, fh * 128:(fh + 1) * 128],
                                rhs=axt[:, k, :],
                                start=(k == 0),
                                stop=(k == 1),
                            )
                        nc.scalar.activation(
                            ht[:, fh, :], ph[:],
                            mybir.ActivationFunctionType.Relu,
                            bias=b1_sb[:, fh, :],
                        )
                    p2b = pt2.tile([128, TCOLS], FP32, tag="t2")
                    for fh in range(2):
                        nc.tensor.matmul(
                            p2b[:],
                            lhsT=ht[:, fh, :],
                            rhs=w2_sb[:, fh, :],
                            start=(fh == 0),
                            stop=(fh == 1),
                        )
                    nc.vector.tensor_copy(adst_sb[:, b: b + 1], p2b[:, 258:259])
                    t2row = t2p.tile([128, TCOLS], BF16, tag="t2r")
                    nc.vector.tensor_tensor(
                        t2row[:], p2b[:], ones_sb[:], op=ad
                    )
                    h, r = divmod(b, hblk)
                    nc.sync.dma_start(
                        t2s_d[h, r * 128:(r + 1) * 128, :], t2row[:]
                    )
                if nblk > hblk:
                    allgather(1)

            # ---------------- layer 2: GAT (two passes by src chunk) -------
            with (
                tc.tile_pool(name="g2_p", bufs=3) as g2p,
                tc.tile_pool(name="p2s_p", bufs=3) as p2p,
                tc.tile_pool(name="pts_p", bufs=3) as ptp,
                tc.tile_pool(name="sc_p", bufs=3) as scp,
                tc.tile_pool(name="m2_p", bufs=4) as m2p,
                tc.tile_pool(name="o_p", bufs=2) as op_,
                tc.psum_pool(name="pdv_p", bufs=2) as pdv,
                tc.psum_pool(name="pag_p", bufs=2) as pag,
            ):
                for ps_ in range(2):
                    Tp = TA if ps_ == 0 else TB
                    for b in range(nblk):
                        tb = Tp[b]
                        o = int(offs2[ps_ * nblk + b])
                        g2 = g2p.tile([128, tb, TCOLS], BF16, tag="g2")
                        gather(g2, t2chunk[ps_], idxs2_sb, tb, o, TCOLS)
                        p2t = p2p.tile([128, tb, 128], BF16, tag="p2")
                        nc.sync.dma_start(p2t[:], p2_d[:, o: o + tb, :])
                        ptt = ptp.tile([128, tb, 128], BF16, tag="pt")
                        nc.sync.dma_start(ptt[:], pt_d[:, o: o + tb, :])
                        # per-edge adst via transposed one-hot matmuls
                        dv = pdv.tile([128, tb], FP32, tag="dv")
                        for t in range(tb):
                            nc.tensor.matmul(
                                dv[:, t: t + 1],
                                lhsT=ptt[:, t, :],
                                rhs=adst_sb[:, b: b + 1],
                                start=True,
                                stop=True,
                            )
                        # alpha = exp(leaky_relu(asrc_src + adst_dst, 0.2))
                        t0 = scp.tile([128, tb], FP32, tag="t0")
                        nc.vector.tensor_tensor(
                            t0[:], g2[:, :, 256], dv[:], op=ad
                        )
                        e = scp.tile([128, tb], FP32, tag="e")
                        nc.vector.scalar_tensor_tensor(
                            e[:], t0[:], 0.2, t0[:], op0=mu, op1=mx
                        )
                        nc.vector.tensor_scalar_min(e[:], e[:], 60.0)
                        al = scp.tile([128, tb], FP32, tag="al")
                        nc.scalar.activation(
                            al[:], e[:], mybir.ActivationFunctionType.Exp
                        )
                        ps = pag.tile([128, 258], FP32, tag="agg")
                        for t in range(tb):
                            m2 = m2p.tile([128, 128], BF16, tag="m2")
                            nc.scalar.mul(m2[:], p2t[:, t, :], al[:, t: t + 1])
                            nc.tensor.matmul(
                                ps[:],
                                lhsT=m2[:],
                                rhs=g2[:, t, 0:258],
                                start=(t == 0),
                                stop=(t == tb - 1),
                            )
                        if ps_ == 0:
                            nc.vector.tensor_copy(partA[:, b, :], ps[:])
                            continue
                        num = op_.tile([128, 258], FP32, tag="num")
                        nc.vector.tensor_tensor(
                            num[:], ps[:], partA[:, b, :], op=ad
                        )
                        sden = scp.tile([128, 1], FP32, tag="sden")
                        nc.vector.tensor_scalar_add(
                            sden[:], num[:, 257:258], 1e-16
                        )
                        rc = scp.tile([128, 1], FP32, tag="rc")
                        nc.vector.reciprocal(rc[:], sden[:])
                        ob = op_.tile([128, D], FP32, tag="ob")
                        nc.vector.scalar_tensor_tensor(
                            ob[:], num[:, 0:D], rc[:], b2_sb[:], op0=mu, op1=ad
                        )
                        nc.vector.tensor_scalar_max(ob[:], ob[:], 0.0)
                        nc.sync.dma_start(
                            out_d[b * 128:(b + 1) * 128, :], ob[:]
                        )
    nc.finalize()
    return nc


# ----------------------------------------------------------------------------
# entry point
# ----------------------------------------------------------------------------

_CACHE = {}


def _get_nc(T1, TA, TB, npad, per, nblk):
    key = (tuple(T1), tuple(TA), tuple(TB), npad, per, nblk)
    if key not in _CACHE:
        _CACHE[key] = _build_nc(T1, TA, TB, npad, per, nblk)
    return _CACHE[key]


def kernel(event_emb, edge_index, W1, b1, W2, att_src, att_dst, b2,
           _want_results=False, _trace=False):
    shared, per_core, (T1, TA, TB), n, npad, per, nblk = _prep(
        event_emb, edge_index, W1, b1, W2, att_src, att_dst, b2
    )
    nc = _get_nc(T1, TA, TB, npad, per, nblk)
    in_maps = [{**shared, **per_core[c]} for c in range(N_CORES)]
    res = run_bass_kernel_spmd(
        nc, in_maps, core_ids=list(range(N_CORES)), trace=_trace
    )
    out = np.concatenate(
        [res.results[c]["out_slice"] for c in range(N_CORES)], axis=0
    )[:n]
    if _want_results:
        return out, res
    return out
